# revision 12
# baseline (speedup 1.0000x reference)
"""Trainium2 Bass kernel for nn_MultiHeadAttention (B=2, S=2048, D=1024, H=16, HD=64).

Strategy (8 NeuronCores, tensor-parallel over heads):
  - Each core owns 2 heads (128 of the 1024 q/k/v features).
  - QKV projections in transposed layout (features on partitions, tokens on
    free dim) from a host-pre-transposed bf16 copy of h.
  - k-bias cancels exactly in softmax (per-query constant score shift) and
    v-bias folds into the output bias (bo' = bo + Wo @ bv), so only bq is
    applied on-device.
  - Attention as S^T = K @ Q^T tiles ([key, query] layout); softmax without
    max-subtraction (scores provably tiny); denominator from a ones-column
    appended to V; causal masking via block skipping + triangular bf16 mask
    multiply on diagonal blocks.
  - Token ownership is STRIPED across the four 1024-token chunks: core c owns
    tokens [1024*m + 128*c, +128) for m=0..3.  After each chunk's attention
    the per-head output is normalised (1/den broadcast via a tiny
    sel-matmul), staged with a single transposed-AP DMA, exchanged with a
    per-chunk AllToAll, unpacked with one DMA, and W_O for that token slice
    runs ~2 chunks later as PE filler — so collectives and W_O fully overlap
    the next chunks' attention.
  - Emission is software-pipelined with a filler deque: projection /W_O /
    normalise pieces are pumped between attention tiles so the PE never
    head-of-line blocks on the Activation engine's exp.
All matmuls bf16 with fp32 PSUM accumulation.
"""

from collections import deque

import numpy as np
import ml_dtypes

import concourse.bass as bass
import concourse.tile as tile
import concourse.mybir as mybir
from concourse import bacc
from concourse.bass_utils import run_bass_kernel_spmd

BF16 = ml_dtypes.bfloat16
F32 = np.float32

B, S, D, H, HD = 2, 2048, 1024, 16, 64
P = 128                      # partitions
TOK = B * S                  # 4096 flattened tokens
DT = D // P                  # 8 d-tiles
NCORES = 8
HLOC = H // NCORES           # 2 heads per core
TOKC = TOK // NCORES         # 512 tokens owned per core
QC = 1024                    # attention query-chunk width
NCH = TOK // QC              # 4 chunks
KT = P                       # key tile = 128
VW = 80                      # per-ktile stride in v_sb ([64 v | 1 ones | 15 pad])

dt_bf = mybir.dt.bfloat16
dt_f32 = mybir.dt.float32
EXP = mybir.ActivationFunctionType.Exp


def _build_nc(single=False, repeat=1):
    # single=True: no collective (replaced by a DRAM->DRAM copy), for
    # single-core timeline simulation / profiling only.
    # repeat>1: run the whole pipeline N times (for wall-clock benchmarking
    # that amortises the host dispatch overhead).
    nc = bacc.Bacc("TRN2", target_bir_lowering=False, debug=False,
                   num_devices=1 if single else NCORES)

    hT = nc.dram_tensor("hT", [P, DT, TOK], dt_bf, kind="ExternalInput")
    woT = nc.dram_tensor("woT", [P, DT, D], dt_bf, kind="ExternalInput")
    wq = nc.dram_tensor("wq", [P, DT, P], dt_bf, kind="ExternalInput")
    wk = nc.dram_tensor("wk", [P, DT, P], dt_bf, kind="ExternalInput")
    wv = nc.dram_tensor("wv", [P, DT, P], dt_bf, kind="ExternalInput")
    bqd = nc.dram_tensor("bq", [P, 1], dt_f32, kind="ExternalInput")
    bod = nc.dram_tensor("bo", [P, DT], dt_f32, kind="ExternalInput")
    trid = nc.dram_tensor("tri", [P, P], dt_bf, kind="ExternalInput")
    seld = nc.dram_tensor("sel2", [2, P], dt_bf, kind="ExternalInput")
    # out[a2, p, tok] = y[128*a2 + p, tok] for this core's 512 striped tokens
    out_t = nc.dram_tensor("out", [DT, P, TOKC], dt_f32, kind="ExternalOutput")

    NG = NCH * repeat  # global chunk count

    with tile.TileContext(nc) as tc:
        with (
            tc.tile_pool(name="persist", bufs=1) as persist,
            tc.tile_pool(name="pt_pool", bufs=5) as pt_pool,
            tc.tile_pool(name="normp", bufs=2) as normp,
            tc.tile_pool(name="outp", bufs=2) as outp,
            tc.tile_pool(name="denp", bufs=2) as denp,
            tc.tile_pool(name="ps_st", bufs=2, space="PSUM") as ps_st,
            tc.tile_pool(name="ps_ot", bufs=1, space="PSUM") as ps_ot,
            tc.tile_pool(name="ps_pj", bufs=1, space="PSUM") as ps_pj,
            tc.tile_pool(name="ps_wo", bufs=1, space="PSUM") as ps_wo,
            tc.tile_pool(name="dram", bufs=1, space="DRAM") as dram,
        ):
            # ---- resident SBUF tensors -------------------------------------
            hT_sb = persist.tile([P, DT, TOK], dt_bf)
            woT_sb = persist.tile([P, DT, D], dt_bf)
            wq_sb = persist.tile([P, DT, P], dt_bf)
            wk_sb = persist.tile([P, DT, P], dt_bf)
            wv_sb = persist.tile([P, DT, P], dt_bf)
            bq_sb = persist.tile([P, 1], dt_f32)
            bo_sb = persist.tile([P, DT], dt_f32)
            tri_sb = persist.tile([P, P], dt_bf)
            sel2_sb = persist.tile([2, P], dt_bf)
            qT_sb = persist.tile([P, TOK], dt_bf)
            kT_sb = persist.tile([P, TOK], dt_bf)
            vT_sb = persist.tile([P, TOK], dt_bf)
            v_sb = persist.tile([P, B, HLOC, S // KT, VW], dt_bf)
            # per-head unnormalised o^T (rows 0-63) + denominator (row 64)
            oun0_sb = persist.tile([HD + 1, TOK], dt_bf)
            oun1_sb = persist.tile([HD + 1, TOK], dt_bf)
            oT_sb = persist.tile([P, DT, TOKC], dt_bf)

            # constants on the gpsimd (SWDGE) queue so the SP queue's first
            # hT tile isn't stuck behind them
            nc.gpsimd.dma_start(wq_sb[:], wq[:])
            nc.gpsimd.dma_start(wk_sb[:], wk[:])
            nc.gpsimd.dma_start(wv_sb[:], wv[:])
            nc.gpsimd.dma_start(bq_sb[:], bqd[:])
            nc.gpsimd.dma_start(bo_sb[:], bod[:])
            nc.gpsimd.dma_start(tri_sb[:], trid[:])
            nc.gpsimd.dma_start(sel2_sb[:], seld[:])
            nc.vector.memset(v_sb[:, :, :, :, HD:HD + 1], 1.0)

            # two collectives per rep: pair pp ships chunks {2pp, 2pp+1}
            a2a_in = [dram.tile([NCORES, P, 2, KT], dt_bf, name=f"a2a_in{m}")
                      for m in range(2)]
            a2a_out = [dram.tile([NCORES, P, 2, KT], dt_bf, name=f"a2a_out{m}")
                       for m in range(2)]

            PROJ = {"q": (wq_sb, qT_sb), "k": (wk_sb, kT_sb), "v": (wv_sb, vT_sb)}

            def proj_half(ch, which, n):
                # one 512-token half of one of q^T/k^T/v^T for chunk ch
                w_sb, dst = PROJ[which]
                lo = QC * ch + 512 * n
                ps = ps_pj.tile([P, 512], dt_f32, tag="pj")
                for a in range(DT):
                    nc.tensor.matmul(
                        ps[:],
                        w_sb[:, a, :],
                        hT_sb[:, a, lo:lo + 512],
                        start=(a == 0),
                        stop=(a == DT - 1),
                    )
                if which == "q":
                    nc.vector.tensor_scalar_add(
                        out=dst[:, lo:lo + 512], in0=ps[:], scalar1=bq_sb[:])
                else:
                    nc.vector.tensor_copy(out=dst[:, lo:lo + 512], in_=ps[:])

            def v_transpose(idx):
                # v^T chunk -> [token, feature] tiles (8 k-tiles) + ones col.
                b2, c = divmod(idx, 2)
                for hh in range(HLOC):
                    nc.scalar.dma_start_transpose(
                        v_sb[:, b2, hh, 8 * c:8 * c + 8, 0:HD],
                        vT_sb[HD * hh:HD * hh + HD,
                              S * b2 + QC * c:S * b2 + QC * c + QC],
                    )

            def ht_dma(g):
                # hT token-chunk for global chunk g (per-rep reload).
                idx = g % NCH
                if idx == 0:
                    for a in range(DT):  # fine-grained so rep-0 starts fast
                        nc.sync.dma_start(
                            hT_sb[:, a, 0:QC], hT[:, a, 0:QC])
                else:
                    # two 1 MB halves: bounds the SP-queue delay seen by
                    # anything issued behind this prefetch
                    for half in range(2):
                        lo = QC * idx + 512 * half
                        nc.sync.dma_start(
                            hT_sb[:, :, lo:lo + 512], hT[:, :, lo:lo + 512])

            def proj_pieces(g):
                ch = g % NCH
                out = []
                for which in ("q", "k", "v"):
                    for n in range(2):
                        out.append(lambda w=which, n=n: proj_half(ch, w, n))
                out.append(lambda: v_transpose(ch))
                return out

            def wo_pieces(g):
                # W_O for chunk g's 128 owned tokens, reading oT_sb slice.
                mi = g % NCH
                sl = slice(KT * mi, KT * mi + KT)
                o_out = outp.tile([P, DT, KT], dt_f32, tag="oo")
                pieces = []

                def half(h2, o_out=o_out):
                    ps = ps_wo.tile([P, 512], dt_f32, tag="wo")
                    for r in range(4):
                        a2 = 4 * h2 + r
                        for a in range(DT):
                            nc.tensor.matmul(
                                ps[:, 128 * r:128 * r + 128],
                                woT_sb[:, a, 128 * a2:128 * a2 + 128],
                                oT_sb[:, a, sl],
                                start=(a == 0), stop=(a == DT - 1),
                            )
                    for r in range(4):
                        a2 = 4 * h2 + r
                        nc.vector.tensor_scalar_add(
                            out=o_out[:, a2, :],
                            in0=ps[:, 128 * r:128 * r + 128],
                            scalar1=bo_sb[:, a2:a2 + 1],
                        )

                pieces.append(lambda: half(0))
                pieces.append(lambda: half(1))
                pieces.append(lambda: nc.scalar.dma_start(
                    out_t[:, :, sl].transpose([1, 0, 2]), o_out[:]))
                return pieces

            def norm_pieces(g):
                # normalise chunk g's two heads + stage; on the second chunk
                # of a pair also exchange + unpack.
                idx = g % NCH
                pp, cc = divmod(idx, 2)
                b2, c = divmod(idx, 2)
                qb = S * b2 + QC * c
                den2 = denp.tile([2, QC], dt_bf, tag="den")
                rec2 = denp.tile([2, QC], dt_bf, tag="rec")
                # den gather + reciprocal emitted immediately (cheap, off PE).
                # SP queue, but norm_pieces is emitted BEFORE the hT prefetch
                # so the recip -> bc chain isn't stuck behind a 2 MB transfer.
                nc.sync.dma_start(den2[0:1, :], oun0_sb[HD:HD + 1, qb:qb + QC])
                nc.sync.dma_start(den2[1:2, :], oun1_sb[HD:HD + 1, qb:qb + QC])
                with nc.allow_low_precision(reason="softmax denom recip to bf16"):
                    nc.vector.reciprocal(rec2[:], den2[:])

                def bc_stage():
                    bc = ps_st.tile([P, QC], dt_f32, tag="st", name="ps_bc")
                    for n in range(2):
                        nc.tensor.matmul(
                            bc[:, 512 * n:512 * n + 512],
                            sel2_sb[:],
                            rec2[:, 512 * n:512 * n + 512],
                            start=True, stop=True,
                        )
                    norm_t = normp.tile([P, QC], dt_bf, tag="nrm")
                    nc.vector.tensor_mul(
                        norm_t[0:HD, :], oun0_sb[0:HD, qb:qb + QC], bc[0:HD, :])
                    nc.vector.tensor_mul(
                        norm_t[HD:P, :], oun1_sb[0:HD, qb:qb + QC], bc[HD:P, :])
                    nc.scalar.dma_start(
                        a2a_in[pp][:, :, cc, :].transpose([1, 0, 2]), norm_t[:])
                    if cc == 1:
                        if single:
                            nc.sync.dma_start(a2a_out[pp][:], a2a_in[pp][:])
                        else:
                            nc.gpsimd.collective_compute(
                                "AllToAll",
                                mybir.AluOpType.bypass,
                                replica_groups=[list(range(NCORES))],
                                ins=[a2a_in[pp].opt()],
                                outs=[a2a_out[pp].opt()],
                            )
                        nc.sync.dma_start(
                            oT_sb[:, :, 2 * KT * pp:2 * KT * pp + 2 * KT],
                            a2a_out[pp][:, :, :, :].transpose([1, 0, 2, 3]))

                return [bc_stage]

            def attention(g, hh, fillers, budget):
                # causal attention for (chunk g, local head hh); pumps
                # `fillers` between tiles, ~`budget` pieces total.
                idx = g % NCH
                b2, c = divmod(idx, 2)
                fb = HD * hh
                qbase = S * b2 + QC * c
                nkt = (QC // KT) * (c + 1)
                ot = ps_ot.tile([P, QC], dt_f32, tag="ot", name="ps_ot")
                pumped = 0

                def pump(target):
                    nonlocal pumped
                    while pumped < min(target, budget) and fillers:
                        fillers.popleft()()
                        pumped += 1

                def s_tile(t):
                    m = t - (QC // KT) * c  # diag block index if >= 0
                    lo_all = KT * m if m >= 0 else 0
                    st = ps_st.tile([P, QC], dt_f32, tag="st", name="ps_att")
                    for n in range(2):
                        lo = max(512 * n, lo_all)
                        hi = 512 * n + 512
                        if lo >= hi:
                            continue
                        nc.tensor.matmul(
                            st[:, lo:hi],
                            kT_sb[fb:fb + HD,
                                  S * b2 + KT * t:S * b2 + KT * t + KT],
                            qT_sb[fb:fb + HD, qbase + lo:qbase + hi],
                            start=True, stop=True,
                        )
                    pt = pt_pool.tile([P, QC], dt_bf, tag="pt", name="pt")
                    nc.scalar.activation(
                        out=pt[:, lo_all:QC], in_=st[:, lo_all:QC],
                        func=EXP, scale=0.125,
                    )
                    if m >= 0:
                        nc.vector.tensor_mul(
                            pt[:, KT * m:KT * m + KT],
                            pt[:, KT * m:KT * m + KT],
                            tri_sb[:],
                        )
                    return pt, lo_all

                def p_tile(t, pt, lo_all):
                    for n in range(2):
                        lo = max(512 * n, lo_all)
                        hi = 512 * n + 512
                        if lo >= hi:
                            continue
                        last_t = (QC // KT) * c + 3 if n == 0 else nkt - 1
                        nc.tensor.matmul(
                            ot[0:HD + 1, lo:hi],
                            v_sb[:, b2, hh, t, 0:HD + 1],
                            pt[:, lo:hi],
                            start=(t == 0), stop=(t == last_t),
                        )

                pending = None
                for i in range(nkt):
                    cur = s_tile(i)
                    if pending is not None:
                        pump((i * budget) // nkt)
                        p_tile(i - 1, *pending)
                    pending = cur
                pump(budget)
                p_tile(nkt - 1, *pending)
                oun = oun0_sb if hh == 0 else oun1_sb
                nc.vector.tensor_copy(
                    out=oun[:, qbase:qbase + QC], in_=ot[0:HD + 1, :])

            # ---- global chunk ring ----------------------------------------
            ht_dma(0)
            # woT on the ACT queue: 2 MB would delay hT chunk-0 on SP; it's
            # only needed by wo_pieces(0) two chunks in.
            nc.scalar.dma_start(woT_sb[:], woT[:])
            for piece in proj_pieces(0):
                piece()

            for g in range(NG):
                fillers = deque()
                # norm first: its den DMAs + recip must beat the 2 MB hT
                # prefetch onto the SP queue (the bc filler matmul otherwise
                # head-of-line blocks the PE for the transfer time)
                norm_f = norm_pieces(g - 1) if g - 1 >= 0 else []
                if g + 1 < NG:
                    ht_dma(g + 1)
                    fillers.extend(proj_pieces(g + 1))
                fillers.extend(norm_f)
                # W_O lags its pair's collective by >= 1 chunk: the pair
                # {2k, 2k+1} exchange fires during chunk 2k+2, so its two
                # W_O slices run during chunk 2k+3.
                if g % 2 == 1 and g >= 3:
                    fillers.extend(wo_pieces(g - 3))
                    fillers.extend(wo_pieces(g - 2))
                n_f = len(fillers)
                attention(g, 0, fillers, (n_f + 1) // 2)
                attention(g, 1, fillers, n_f)
                while fillers:
                    fillers.popleft()()

            # epilogue: last chunk's exchange + final two W_O slices
            for piece in norm_pieces(NG - 1):
                piece()
            for piece in wo_pieces(NG - 2):
                piece()
            for piece in wo_pieces(NG - 1):
                piece()

    return nc


def _retile(x):
    """[D, N] -> [P, DT, N] with d = a*128 + p."""
    return np.ascontiguousarray(
        x.reshape(DT, P, -1).transpose(1, 0, 2)
    )


def _prepare_inputs(h, Wq, bq, Wk, bk, Wv, bv, Wo, bo):
    h2 = np.asarray(h, dtype=np.float32).reshape(TOK, D)
    hT_t = _retile(h2.T.astype(BF16))
    woT_t = _retile(np.asarray(Wo, np.float32).T.astype(BF16))
    # v-bias folds into the output bias: o = sum(p)*v + bv  (sum p = 1)
    bo2 = np.asarray(bo, np.float32) + np.asarray(Wo, np.float32) @ np.asarray(
        bv, np.float32)
    bo_t = np.ascontiguousarray(bo2.reshape(DT, P).T).astype(np.float32)

    tri = np.triu(np.ones((P, P), np.float32)).astype(BF16)
    sel2 = np.zeros((2, P), np.float32)
    sel2[0, 0:HD] = 1.0
    sel2[1, HD:P] = 1.0
    sel2 = sel2.astype(BF16)

    in_maps = []
    for c in range(NCORES):
        r = slice(P * c, P * c + P)
        in_maps.append(dict(
            hT=hT_t,
            woT=woT_t,
            wq=_retile(np.asarray(Wq, np.float32)[r].T.astype(BF16)),
            wk=_retile(np.asarray(Wk, np.float32)[r].T.astype(BF16)),
            wv=_retile(np.asarray(Wv, np.float32)[r].T.astype(BF16)),
            bq=np.asarray(bq, np.float32)[r].reshape(P, 1).copy(),
            bo=bo_t,
            tri=tri,
            sel2=sel2,
        ))
    return in_maps


def _gather_output(core_outs):
    # core c's out[a2, p, m*128 + t] holds y[128*a2+p, token 1024*m + 128*c + t]
    outT = np.empty((D, TOK), np.float32)
    for c, o in enumerate(core_outs):
        o2 = np.asarray(o).reshape(D, NCH, KT)
        for m in range(NCH):
            outT[:, QC * m + KT * c:QC * m + KT * c + KT] = o2[:, m, :]
    return np.ascontiguousarray(outT.T).reshape(B, S, D).astype(np.float32)


LAST_RESULTS = None  # BassKernelResults of the most recent kernel() call


def kernel(h, Wq, bq, Wk, bk, Wv, bv, Wo, bo):
    global LAST_RESULTS
    in_maps = _prepare_inputs(h, Wq, bq, Wk, bk, Wv, bv, Wo, bo)
    nc = _build_nc()
    nc.compile()
    res = run_bass_kernel_spmd(nc, in_maps, core_ids=list(range(NCORES)))
    LAST_RESULTS = res
    return _gather_output([r["out"] for r in res.results])


if __name__ == "__main__":
    d = np.load("/root/problem/inputs_cache.npz")
    out = kernel(**{k: d[k] for k in d.files})
    print("out", out.shape, out.dtype, np.abs(out).max())
